# revision 53
# baseline (speedup 1.0000x reference)
"""Self-contained Trainium2 Bass kernel for the DrugFEM GAT model.

2-layer GATConv (heads=1, self-loops) + global-softmax attention pooling +
Linear/BatchNorm/LeakyReLU/Linear head.  Runs SPMD on 8 NeuronCores:
nodes (and their in-edges) are partitioned across cores; per-layer node
feature tables are replicated via AllGather; edge aggregation uses per
(window, quartile) indirect DMA gathers with exact valid counts (no pad
descriptors), whole-window broadcast DVE ops for the one-hot selector
build, and selector-matmul accumulation in PSUM.  Self-loop edges are
handled separately with one direct contiguous row load per window.
"""

import os
import sys

import numpy as np

for _p in ("/opt/trn_rl_repo", "/root/.axon_site/_ro/trn_rl_repo"):
    if os.path.isdir(_p) and _p not in sys.path:
        sys.path.insert(0, _p)

import ml_dtypes  # noqa: E402

from concourse import bacc, mybir, tile  # noqa: E402

# Force Exp/Ln/Copy to resolve to the one ACT func-set that contains all of
# them: emptying the narrower exp/ln sets stops bacc's table-load pass from
# thrashing tables (~1.3us per reload) inside the per-window loop.
_orig_get_act_tables = bacc.get_activation_tables


def _patched_get_act_tables(arch):
    tabs = dict(_orig_get_act_tables(arch))
    combined = None
    for name, funcs in tabs.items():
        fl = {str(f) for f in funcs}
        if any("Exp" in s for s in fl) and any("Ln" in s for s in fl):
            combined = name
            break
    if combined is not None:
        for name, funcs in list(tabs.items()):
            if name == combined:
                continue
            fl = {str(f) for f in funcs}
            if any("Exp" in s for s in fl) or any("Ln" in s for s in fl):
                tabs[name] = type(funcs)()
    return tabs


bacc.get_activation_tables = _patched_get_act_tables
from concourse.bass_utils import run_bass_kernel_spmd  # noqa: E402

BF16 = ml_dtypes.bfloat16
F = 128          # feature dim (hidden dim D == F_IN == 128)
P = 128          # partitions
ROWE = 256       # gather-table row, in bf16 elements (= 512 bytes)
Q = 4            # quartile split of the node table (int16 gather indices)
BN_EPS = 1e-5
GAT_SLOPE = 0.2
ACT_SLOPE = 0.01


def _full_cfg():
    return dict(N=100_000, E=1_600_000, G=4096, C=8)


def _r16(x):
    return (int(x) + 15) // 16 * 16


# --------------------------------------------------------------------------
# host-side sharding / metadata
# --------------------------------------------------------------------------

def _host_prep(inputs, cfg):
    N, G, C = cfg["N"], cfg["G"], cfg["C"]
    npc = N // C
    assert npc * C == N, "node count must divide core count"
    W = -(-npc // P)
    npc_pad = W * P
    assert N % Q == 0 and N // Q <= 32768 and npc % Q == 0
    npq = npc // Q

    # quartile 3's table is assembled by TWO half-AllGathers so the first
    # half can fly while the last L1 windows still compute; its row layout
    # is [concat_c shard[0:s3] ; concat_c shard[s3:npq]].
    s3 = min(npq, ((npq // 2) + 15) // 16 * 16)
    s3b = npq - s3

    ei = np.asarray(inputs["edge_index"]).astype(np.int64)
    src_a, dst_a = ei[0], ei[1]
    batch = np.asarray(inputs["batch"]).astype(np.int64)
    x = np.asarray(inputs["x"]).astype(np.float32)
    assert x.shape == (N, F)

    # host-computed layer-1 node table (saves phase A + the T1 AllGather):
    # row r of quartile q holds node (r//npq)*npc + q*npq + (r%npq), laid out
    # as [x_bf16[0:128] | col128=1.0 | cols130:131 = f32 score bytes].
    xw1 = x.astype(np.float64) @ np.asarray(inputs["W1"], np.float64)
    s1 = (xw1 @ np.asarray(inputs["a_src1"], np.float64).reshape(F)
          ).astype(np.float32)
    d1 = (xw1 @ np.asarray(inputs["a_dst1"], np.float64).reshape(F)
          ).astype(np.float32)
    Nq_ = N // Q

    def _t1_rows(nodes):
        n = len(nodes)
        rows = np.zeros((n, ROWE), BF16)
        rows[:, :F] = x[nodes].astype(BF16)
        rows[:, F] = 1.0
        bv = rows.view(np.uint8).reshape(n, 2 * ROWE)
        bv[:, 260:264] = s1[nodes].astype("<f4").view(np.uint8).reshape(-1, 4)
        return rows

    T1q_host = []
    for q in range(Q):
        r = np.arange(Nq_)
        if q == 3 and s3 < npq:
            in_h1 = r >= C * s3
            rr = np.where(in_h1, r - C * s3, r)
            sz = np.where(in_h1, s3b, s3)
            nodes = ((rr // sz) * npc + q * npq
                     + np.where(in_h1, s3 + rr % sz, rr % sz))
        else:
            nodes = (r // npq) * npc + q * npq + (r % npq)
        T1q_host.append(_t1_rows(nodes))
    assert np.bincount(batch, minlength=G).max() <= P, "graph > 128 nodes"

    order = np.argsort(dst_a, kind="stable")
    src_s, dst_s = src_a[order], dst_a[order]
    owner = dst_s // npc
    core_start = np.searchsorted(owner, np.arange(C + 1))

    # per-core edge partitions with (w, q) keys
    per_core = []
    cnt = np.zeros((C, W, Q), np.int64)
    for c in range(C):
        lo, hi = core_start[c], core_start[c + 1]
        s_c, d_c = src_s[lo:hi], dst_s[lo:hi]
        loc = d_c - c * npc
        w_c = loc // P
        q_c = (s_c % npc) // npq
        key = w_c * Q + q_c
        korder = np.argsort(key, kind="stable")
        s_c, loc, w_c, q_c = (a[korder] for a in (s_c, loc, w_c, q_c))
        key = key[korder]
        kstart = np.searchsorted(key, np.arange(W * Q + 1))
        np.add.at(cnt[c], (w_c, q_c), 1)
        per_core.append((s_c, loc, kstart))

    # shared (cross-core max) per-(w,q) sizes so the single SPMD program
    # fits every core; per-core surplus slots use idx 0 with mask 0.
    # Rounded to full 128-slot chunks so the gather writes every slot the
    # aggregation matmul reads (no per-window SBUF memsets needed).
    nidx = np.zeros((W, Q), np.int64)
    for w in range(W):
        for q in range(Q):
            c_max = int(cnt[:, w, q].max())
            nidx[w, q] = -(-c_max // P) * P if c_max else 0
    KWq = -(-nidx // P)
    qoff = np.zeros((W, Q), np.int64)
    qoff[:, 1:] = np.cumsum(KWq, axis=1)[:, :-1]
    KW_w = KWq.sum(axis=1)
    KWmax = int(KW_w.max())
    ioff = np.zeros((W, Q), np.int64)
    ioff[:, 1:] = np.cumsum(nidx // 16, axis=1)[:, :-1]
    IDXW = int((nidx // 16).sum(axis=1).max())
    rowb = 6 * KWmax

    # self-loop row segments per window: (q, row0_in_q, a, b)
    selfsegs = []
    for w in range(W):
        r0 = w * P
        rows = min(P, npc - r0)
        segs = []
        r = r0
        while r < r0 + rows:
            q = r // npq
            r_end = min((q + 1) * npq, r0 + rows)
            segs.append((int(q), int(r - q * npq), int(r - r0),
                         int(r_end - r0)))
            r = r_end
        selfsegs.append(segs)

    cfg = dict(cfg, npc=npc, W=W, npc_pad=npc_pad, npq=npq, KWmax=KWmax,
               IDXW=IDXW, rowb=rowb, s3=s3,
               nidx=tuple(map(tuple, nidx)), KWq=tuple(map(tuple, KWq)),
               qoff=tuple(map(tuple, qoff)), ioff=tuple(map(tuple, ioff)),
               KW_w=tuple(int(k) for k in KW_w),
               selfsegs=tuple(tuple(s) for s in selfsegs))

    in_maps = []
    for c in range(C):
        s_c, loc, kstart = per_core[c]
        gidx = np.zeros((W, 16, IDXW), np.int16)
        dstl = np.full((W, P, KWmax), 127.0, np.float32)
        mask = np.zeros((W, P, KWmax), np.float32)
        for w in range(W):
            for q in range(Q):
                a, b = kstart[w * Q + q], kstart[w * Q + q + 1]
                n_e = b - a
                nx = int(nidx[w, q])
                if nx == 0:
                    continue
                srcs = s_c[a:b]
                if q == 3 and s3 < npq:
                    i_in_q = (srcs % npc) % npq
                    tabrow = np.where(
                        i_in_q < s3,
                        (srcs // npc) * s3 + i_in_q,
                        C * s3 + (srcs // npc) * s3b + (i_in_q - s3))
                else:
                    tabrow = (srcs // npc) * npq + (srcs % npc) % npq
                vals = np.zeros(nx, np.int64)
                vals[:n_e] = tabrow
                wr = vals.reshape(nx // 16, 16).T.astype(np.int16)
                gidx[w, :, ioff[w, q]:ioff[w, q] + nx // 16] = wr
                i_arr = np.arange(n_e)
                cc = qoff[w, q] + i_arr // P
                pp = i_arr % P
                dstl[w, pp, cc] = (loc[a:b] - w * P).astype(np.float32)
                mask[w, pp, cc] = 1.0
        gidx = np.tile(gidx, (1, 8, 1))
        meta = np.zeros((W, P, rowb), np.uint8)
        meta[:, :, 0:2 * KWmax] = (
            dstl.astype(BF16).view(np.uint8).reshape(W, P, -1))
        meta[:, :, 2 * KWmax:6 * KWmax] = (
            mask.view(np.uint8).reshape(W, P, -1))

        # pooling metadata (unchanged from baseline)
        b_loc = batch[c * npc:(c + 1) * npc]
        brel = np.full((P, W), 500.0, np.float32)
        nmask = np.zeros((P, W), np.float32)
        scat = np.zeros((P, 8 * W), np.int16)
        GP = G + P
        for w in range(W):
            n0 = w * P
            n1 = min(n0 + P, npc)
            base = int(b_loc[n0])
            span = int(b_loc[n1 - 1]) - base + 1
            brel[: n1 - n0, w] = (b_loc[n0:n1] - base).astype(np.float32)
            nmask[: n1 - n0, w] = 1.0
            par = (w % 2) * GP
            idx = np.where(np.arange(P) < span,
                           par + base + np.arange(P),
                           par + G + np.arange(P)).astype(np.int16)
            wrapped = idx.reshape(8, 16).T
            scat[:, 8 * w:8 * w + 8] = np.tile(wrapped, (8, 1))

        d1_loc = np.zeros(npc_pad, np.float32)
        d1_loc[:npc] = d1[c * npc:(c + 1) * npc]
        m = dict(
            meta=meta,
            gidx=gidx,
            brel=brel,
            nmask=nmask,
            scat=scat,
            d1=d1_loc.reshape(W, P).T.copy(),
        )
        for q in range(Q):
            m[f"T1q{q}"] = T1q_host[q]
            m[f"self1q{q}"] = _t1_rows(
                c * npc + q * npq + np.arange(npq))
        m.update(
            W1=np.asarray(inputs["W1"], np.float32),
            W2=np.asarray(inputs["W2"], np.float32),
            a_src2=np.asarray(inputs["a_src2"], np.float32).reshape(F, 1),
            a_dst2=np.asarray(inputs["a_dst2"], np.float32).reshape(F, 1),
            b1=np.asarray(inputs["b1"], np.float32).reshape(1, F),
            b2=np.asarray(inputs["b2"], np.float32).reshape(1, F),
            w_attn=np.asarray(inputs["w_attn"], np.float32).reshape(F, 1),
            b_attn=np.asarray(inputs["b_attn"], np.float32).reshape(1, 1),
            fc_w1=np.asarray(inputs["fc_w1"], np.float32),
            fc_b1=np.asarray(inputs["fc_b1"], np.float32).reshape(-1, 1),
            bn_g=np.asarray(inputs["bn_g"], np.float32).reshape(-1, 1),
            bn_b=np.asarray(inputs["bn_b"], np.float32).reshape(-1, 1),
            fc_w2=np.asarray(inputs["fc_w2"], np.float32),
            fc_b2=np.asarray(inputs["fc_b2"], np.float32).reshape(-1, 1),
            iota_b=np.tile(np.arange(P, dtype=np.float32), (P, 1)).astype(BF16),
            iota_f=np.tile(np.arange(P, dtype=np.float32), (P, 1)),
            ident=np.eye(P, dtype=np.float32),
            ones_c=np.ones((P, 1), np.float32),
            ones_r=np.ones((1, P), np.float32),
        )
        in_maps.append(m)
    return in_maps, cfg


# --------------------------------------------------------------------------
# device program
# --------------------------------------------------------------------------

def _build_program(cfg):
    N, G, C = cfg["N"], cfg["G"], cfg["C"]
    W = cfg["W"]
    H = cfg.get("Dh", 64)          # hidden dim of the fc head (D//2)
    TG = G // P                    # graph tiles in the head
    assert G % P == 0
    GP = G + P
    rowb = cfg["rowb"]

    f32 = mybir.dt.float32
    bf16 = mybir.dt.bfloat16
    i16 = mybir.dt.int16
    u8 = mybir.dt.uint8
    Op = mybir.AluOpType
    Act = mybir.ActivationFunctionType

    nc = bacc.Bacc("TRN2", target_bir_lowering=False, debug=False,
                   enable_asserts=False, num_devices=C,
                   num_swdge_queues=4, dynamic_dma_scratch_size=65536)

    def din(name, shape, dt):
        return nc.dram_tensor(name, shape, dt, kind="ExternalInput")

    Nq = N // 4
    npq = cfg["npq"]
    t = {}
    t["meta"] = din("meta", [W, P, rowb], u8)
    t["gidx"] = din("gidx", [W, P, cfg["IDXW"]], i16)
    t["brel"] = din("brel", [P, W], f32)
    t["nmask"] = din("nmask", [P, W], f32)
    t["scat"] = din("scat", [P, 8 * W], i16)
    t["d1"] = din("d1", [P, W], f32)
    for q in range(4):
        t[f"T1q{q}"] = din(f"T1q{q}", [Nq, ROWE], bf16)
        t[f"self1q{q}"] = din(f"self1q{q}", [npq, ROWE], bf16)
    for nm, sh in [("W1", [F, F]), ("W2", [F, F]), ("fc_w1", [F, H]),
                   ("fc_w2", [H, F]),
                   ("a_src2", [F, 1]), ("a_dst2", [F, 1]), ("b1", [1, F]),
                   ("b2", [1, F]), ("w_attn", [F, 1]), ("b_attn", [1, 1]),
                   ("fc_b1", [H, 1]), ("bn_g", [H, 1]), ("bn_b", [H, 1]),
                   ("fc_b2", [F, 1]), ("iota_f", [P, P]), ("ident", [P, P]),
                   ("ones_c", [P, 1]), ("ones_r", [1, P])]:
        t[nm] = din(nm, sh, f32)
    t["iota_b"] = din("iota_b", [P, P], bf16)
    out_d = nc.dram_tensor("out", [G, F], f32, kind="ExternalOutput")
    if cfg.get("dbg"):
        KWmax = cfg["KWmax"]
        for nm, sh, dt in [("dbg_gsl", [P, KWmax * ROWE], bf16),
                           ("dbg_cmpb", [P, KWmax * P], bf16),
                           ("dbg_dvec", [P, KWmax], f32),
                           ("dbg_logit", [P, KWmax], f32),
                           ("dbg_wcol", [P, KWmax], f32),
                           ("dbg_num", [P, F + 1], f32),
                           ("dbg_den", [P, 1], f32),
                           ("dbg_selfr", [P, ROWE], bf16),
                           ("dbg_wself", [P, 1], f32),
                           ("dbg_hv", [P, F], bf16)]:
            t[nm] = nc.dram_tensor(nm, sh, dt, kind="ExternalOutput")

    with tile.TileContext(nc) as tc:
        _emit(nc, tc, t, out_d, cfg, H, TG, GP, rowb,
              f32, bf16, i16, Op, Act)
    nc.compile()
    return nc


def _emit(nc, tc, t, out_d, cfg, H, TG, GP, rowb, f32, bf16, i16, Op, Act):
    PH = cfg.get("phases", "AL1L2PT")
    N, G, C = cfg["N"], cfg["G"], cfg["C"]
    npc, W = cfg["npc"], cfg["W"]
    npq = cfg["npq"]
    KWmax, IDXW = cfg["KWmax"], cfg["IDXW"]
    nidx, KWq, qoff, ioff = cfg["nidx"], cfg["KWq"], cfg["qoff"], cfg["ioff"]
    KW_w, selfsegs = cfg["KW_w"], cfg["selfsegs"]
    Nq = N // Q
    rg = [list(range(C))]

    def sb(name, shape, dt):
        return nc.alloc_sbuf_tensor(name, list(shape), dt).ap()

    # ---- persistent sbuf ----
    iota_f = sb("iota_f_s", [P, P], f32)
    iota_b = sb("iota_b_s", [P, P], bf16)
    ident = sb("ident_s", [P, P], f32)
    ones_c = sb("ones_c_s", [P, 1], f32)
    ones_r = sb("ones_r_s", [1, P], f32)
    W1sb = sb("W1_s", [P, F], f32)
    W2sb = sb("W2_s", [P, F], f32)
    fw1 = sb("fw1_s", [F, H], f32)
    fw2 = sb("fw2_s", [H, F], f32)
    fb1 = sb("fb1_s", [H, 1], f32)
    bng = sb("bng_s", [H, 1], f32)
    bnb = sb("bnb_s", [H, 1], f32)
    fb2 = sb("fb2_s", [F, 1], f32)
    a_cols = {nm: sb(nm + "_s", [F, 1], f32)
              for nm in ("a_src2", "a_dst2", "w_attn")}
    b_rows = {nm: sb(nm + "_s", [1, F], f32) for nm in ("b1", "b2")}
    battn = sb("battn_s", [1, 1], f32)

    a_reps = {nm: sb(nm + "_rep", [P, P], f32)
              for nm in ("a_src2", "a_dst2", "w_attn")}
    b_reps = {nm: sb(nm + "_rep", [P, P], f32) for nm in ("b1", "b2")}
    battn_c = sb("battn_col", [P, 1], f32)

    d_all = [sb("d1_all", [P, W], f32), sb("d2_all", [P, W], f32)]
    wn_all = sb("wn_all", [P, W], f32)
    h2st = [sb("h2st_a", [P, 132], f32), sb("h2st_b", [P, 132], f32)]
    stage = [sb("stage_a", [P, ROWE], bf16), sb("stage_b", [P, ROWE], bf16)]
    scat_st = [sb("scat_a", [P, 192], f32), sb("scat_b", [P, 192], f32)]
    trash_f = sb("trash_f", [P, P], f32)
    n_zrow = 2 * GP * 192 // P // 192          # pooled rows per partition
    zchunk = min(n_zrow, 11)
    zero_sb = sb("zero_sb", [P, zchunk * 192], f32)
    z_all = sb("z_all", [H, G], f32)
    trash_z = sb("trash_z", [H, P], f32)

    brel_sb = sb("brel_s", [P, W], f32)
    nmask_sb = sb("nmask_s", [P, W], f32)
    scat_sb = sb("scat_s", [P, 8 * W], i16)

    v = nc.vector
    s_ = nc.scalar
    pe = nc.tensor
    gp = nc.gpsimd
    sy = nc.sync

    # ---- dram scratch ----
    # collective OUTPUTS live in Shared (pair-HBM) address space — the fast
    # path for HBM-HBM collectives; inputs must stay Local.
    T2 = [nc.dram_tensor(f"T2q{q}", [Nq, ROWE], bf16, kind="Internal",
                         addr_space="Shared").ap() for q in range(Q)]
    pooled_r = nc.dram_tensor("pooled_r", [2 * GP, 192], f32, kind="Internal",
                              addr_space="Shared").ap()
    with tc.tile_pool(name="dram", bufs=1, space="DRAM") as dpool:
        cc2 = [dpool.tile([npq, ROWE], bf16, name=f"cc2q{q}") for q in range(Q)]
        pooled = dpool.tile([2 * GP, 192], f32)

        with (
            tc.tile_pool(name="meta", bufs=3) as meta_pool,
            tc.tile_pool(name="gath", bufs=3) as gath_pool,
            tc.tile_pool(name="selfp", bufs=2) as self_pool,
            tc.tile_pool(name="cmp", bufs=2) as cmp_pool,
            tc.tile_pool(name="cols", bufs=2) as cols_pool,
            tc.tile_pool(name="fl", bufs=2) as fl_pool,
            tc.tile_pool(name="xw", bufs=2) as xw_pool,
            tc.tile_pool(name="tail", bufs=2) as tail_pool,
            tc.tile_pool(name="ps", bufs=2, space="PSUM") as ps_pool,
            tc.tile_pool(name="psm", bufs=2, space="PSUM") as psm_pool,
            tc.tile_pool(name="pst", bufs=2, space="PSUM") as pst_pool,
        ):
            # ================= prologue =================
            for nm, dest in [("iota_f", iota_f), ("iota_b", iota_b),
                             ("ident", ident), ("ones_c", ones_c),
                             ("ones_r", ones_r), ("W1", W1sb), ("W2", W2sb),
                             ("fc_w1", fw1), ("fc_w2", fw2), ("fc_b1", fb1),
                             ("bn_g", bng), ("bn_b", bnb), ("fc_b2", fb2),
                             ("brel", brel_sb), ("nmask", nmask_sb),
                             ("scat", scat_sb), ("b_attn", battn),
                             ("d1", d_all[0])]:
                sy.dma_start(dest, t[nm].ap())
            for nm in a_cols:
                sy.dma_start(a_cols[nm], t[nm].ap())
            for nm in b_rows:
                sy.dma_start(b_rows[nm], t[nm].ap())

            for hs in h2st:
                v.memset(hs[:, F:132], 1.0)
            v.memset(zero_sb, 0.0)
            for st in stage:
                v.memset(st[:, F:ROWE], 0.0)
                v.memset(st[:, F:F + 1], 1.0)
            for st in scat_st:
                v.memset(st, 0.0)
            # pre-fill the gather-pool ring once so per-window partial-chunk
            # memsets are unnecessary (stale finite data is masked out).
            for _i in range(3):
                g0 = gath_pool.tile([P, KWmax, ROWE], bf16, tag="gsl")
                v.memset(g0[:], 0.0)

            def rep_from_row(row_ap, dest):
                ps = psm_pool.tile([P, P], f32, tag="prep")
                pe.matmul(out=ps[:], lhsT=ones_r, rhs=row_ap, start=True,
                          stop=True)
                v.tensor_copy(dest, ps[:])

            def rep_from_col(col_ap, dest):
                psr = pst_pool.tile([1, P], f32, tag="ptiny")
                pe.transpose(out=psr[:], in_=col_ap, identity=ident)
                row = xw_pool.tile([1, P], f32, tag="prowsb")
                v.tensor_copy(row[:], psr[:])
                rep_from_row(row[:], dest)

            # W transposes for effective attention vectors
            WT = {}
            for nm, wsb in (("W2", W2sb),):
                pst = psm_pool.tile([P, P], f32, tag="prep")
                pe.transpose(out=pst[:], in_=wsb, identity=ident)
                wt = sb(nm + "T_s", [P, F], f32)
                v.tensor_copy(wt, pst[:])
                WT[nm] = wt
            for nm, wnm in (("a_src2", "W2"), ("a_dst2", "W2")):
                pse = pst_pool.tile([1, P], f32, tag="ptiny")
                pe.matmul(out=pse[:], lhsT=a_cols[nm], rhs=WT[wnm],
                          start=True, stop=True)
                row = xw_pool.tile([1, P], f32, tag="prowsb")
                v.tensor_copy(row[:], pse[:])
                rep_from_row(row[:], a_reps[nm])
            rep_from_col(a_cols["w_attn"], a_reps["w_attn"])
            for nm in b_rows:
                rep_from_row(b_rows[nm], b_reps[nm])
            psb = pst_pool.tile([P, 1], f32, tag="ptiny")
            pe.matmul(out=psb[:], lhsT=ones_r, rhs=battn, start=True,
                      stop=True)
            v.tensor_copy(battn_c, psb[:])

            # zero the pooled accumulators
            pv = pooled[:].rearrange("(a p) e -> p a e", p=P)
            zv = zero_sb.rearrange("p (a e) -> p a e", e=192)
            a0 = 0
            while a0 < n_zrow:
                a1 = min(a0 + zchunk, n_zrow)
                sy.dma_start(pv[:, a0:a1, :], zv[:, 0:a1 - a0, :])
                a0 = a1

            def shard_write(ccq, st, w):
                r0 = w * P
                rows = min(P, npc - r0)
                a = 0
                while a < rows:
                    q = (r0 + a) // npq
                    b = min(rows, (q + 1) * npq - r0)
                    sy.dma_start(ccq[q][r0 + a - q * npq:r0 + b - q * npq, :],
                                 st[a:b, :])
                    a = b

            # fire each quartile's AllGather as soon as its shard rows are
            # written; quartile 3 goes as two half-AllGathers so only the
            # second half's transfer is exposed after the last window.
            s3 = cfg["s3"]
            ag_after = {}
            for q in range(3):
                ag_after.setdefault(min(((q + 1) * npq - 1) // P, W - 1),
                                    []).append((q, 0, npq, 0))
            if s3 < npq:
                ag_after.setdefault(min((3 * npq + s3 - 1) // P, W - 1),
                                    []).append((3, 0, s3, 0))
                ag_after.setdefault(W - 1, []).append((3, s3, npq, C * s3))
            else:
                ag_after.setdefault(W - 1, []).append((3, 0, npq, 0))

            def ag_fire(ccq, Tq, q, a, b, o):
                if C == 1:
                    gp.dma_start(Tq[q][o:o + (b - a), :], ccq[q][a:b, :])
                else:
                    gp.collective_compute(
                        "AllGather", Op.bypass, replica_groups=rg,
                        ins=[ccq[q][a:b, :].opt()],
                        outs=[Tq[q][o:o + C * (b - a), :].opt()])

            # layer-1 table + self rows + d1 come precomputed from the host
            T1 = [t[f"T1q{q}"].ap() for q in range(Q)]
            self1 = [t[f"self1q{q}"].ap() for q in range(Q)]

            # ================= GAT layers =================
            def gat_layer(li, Tfull, ccin, ccout, Wsb, brep, d_this, d_next,
                          a_s2, a_d2):
                for w in range(W):
                    KW = KW_w[w]
                    meta = meta_pool.tile([P, rowb], mybir.dt.uint8,
                                          tag="meta")
                    sy.dma_start(meta[:], t["meta"].ap()[w])
                    dstv = meta[:, 0:2 * KWmax].bitcast(bf16)
                    maskv = meta[:, 2 * KWmax:6 * KWmax].bitcast(f32)
                    idxt = meta_pool.tile([P, IDXW], i16, tag="idxt")
                    sy.dma_start(idxt[:], t["gidx"].ap()[w])

                    gsl = gath_pool.tile([P, KWmax, ROWE], bf16, tag="gsl")
                    for q in range(Q):
                        nx = nidx[w][q]
                        if nx == 0:
                            continue
                        kq = KWq[w][q]
                        gp.dma_gather(
                            gsl[:, qoff[w][q]:qoff[w][q] + kq, :],
                            Tfull[q],
                            idxt[:, ioff[w][q]:ioff[w][q] + nx // 16],
                            nx, nx, ROWE, single_packet=False,
                            queue_num=q)

                    # self-loop rows (contiguous in the input cc tables)
                    selfr = self_pool.tile([P, ROWE], bf16, tag="selfr")
                    if selfsegs[w][-1][3] < P:
                        v.memset(selfr[:], 0.0)
                    for (q, r0q, a, b) in selfsegs[w]:
                        sy.dma_start(selfr[a:b, :], ccin[q][r0q:r0q + b - a, :])

                    # d broadcast [P, P]: row j = d_this[j, w]
                    psr = pst_pool.tile([1, P], f32, tag="ptiny")
                    pe.transpose(out=psr[:], in_=d_this[:, w:w + 1],
                                 identity=ident)
                    drow = xw_pool.tile([1, P], f32, tag="prowsb")
                    v.tensor_copy(drow[:], psr[:])
                    psd = psm_pool.tile([P, P], f32, tag="prep")
                    pe.matmul(out=psd[:], lhsT=ones_r, rhs=drow[:],
                              start=True, stop=True)
                    drep = fl_pool.tile([P, P], bf16, tag="drep")
                    s_.activation(out=drep[:], in_=psd[:], func=Act.Copy)

                    # whole-window selector build
                    cmpb = cmp_pool.tile([P, KWmax * P], bf16, tag="cmpb")
                    cmp3 = cmpb[:, 0:KW * P].rearrange("p (k j) -> p k j",
                                                       k=KW)
                    io_b = iota_b.unsqueeze(1).broadcast_to([P, KW, P])
                    dv_b = dstv[:, 0:KW].unsqueeze(2).broadcast_to([P, KW, P])
                    v.tensor_tensor(out=cmp3, in0=dv_b, in1=io_b,
                                    op=Op.is_equal)
                    tmpb = cmp_pool.tile([P, KWmax * P], bf16, tag="tmpb")
                    tmp3 = tmpb[:, 0:KW * P].rearrange("p (k j) -> p k j",
                                                       k=KW)
                    dr_b = drep[:].unsqueeze(1).broadcast_to([P, KW, P])
                    v.tensor_tensor(out=tmp3, in0=cmp3, in1=dr_b, op=Op.mult)
                    dvec = cols_pool.tile([P, KWmax], f32, tag="dvec")
                    v.tensor_reduce(out=dvec[:, 0:KW], in_=tmp3,
                                    axis=mybir.AxisListType.X, op=Op.add)

                    # per-edge logit -> weight
                    sview = gsl[:, 0:KW, 130:132].bitcast(f32).squeeze(2)
                    logit = cols_pool.tile([P, KWmax], f32, tag="logit")
                    v.tensor_tensor(out=logit[:, 0:KW], in0=dvec[:, 0:KW],
                                    in1=sview, op=Op.add)
                    v.scalar_tensor_tensor(out=logit[:, 0:KW],
                                           in0=logit[:, 0:KW],
                                           scalar=GAT_SLOPE,
                                           in1=logit[:, 0:KW],
                                           op0=Op.mult, op1=Op.max)
                    # padded slots carry real row-0 scores (bounded), so exp
                    # stays finite; the post-exp mask zeroes their weight.
                    wcol = cols_pool.tile([P, KWmax], f32, tag="wcol")
                    s_.activation(out=wcol[:, 0:KW], in_=logit[:, 0:KW],
                                  func=Act.Exp)
                    v.tensor_tensor(out=wcol[:, 0:KW], in0=wcol[:, 0:KW],
                                    in1=maskv[:, 0:KW], op=Op.mult)

                    swt = cmp_pool.tile([P, KWmax * P], bf16, tag="swt")
                    sw3 = swt[:, 0:KW * P].rearrange("p (k j) -> p k j", k=KW)
                    wc_b = wcol[:, 0:KW].unsqueeze(2).broadcast_to([P, KW, P])
                    v.tensor_tensor(out=sw3, in0=wc_b, in1=cmp3, op=Op.mult)

                    # aggregation
                    psw = ps_pool.tile([P, F + 1], f32, tag="psw")
                    for cc in range(KW):
                        pe.matmul(out=psw[:], lhsT=swt[:, cc * P:(cc + 1) * P],
                                  rhs=gsl[:, cc, 0:F + 1],
                                  start=(cc == 0), stop=(cc == KW - 1))
                    num = fl_pool.tile([P, F + 1], f32, tag="num")
                    s_.activation(out=num[:], in_=psw[:], func=Act.Copy)

                    # self-loop contribution
                    wself = cols_pool.tile([P, 1], f32, tag="wself")
                    v.tensor_tensor(out=wself[:],
                                    in0=selfr[:, 130:132].bitcast(f32),
                                    in1=d_this[:, w:w + 1], op=Op.add)
                    v.scalar_tensor_tensor(out=wself[:], in0=wself[:],
                                           scalar=GAT_SLOPE, in1=wself[:],
                                           op0=Op.mult, op1=Op.max)
                    s_.activation(out=wself[:], in_=wself[:], func=Act.Exp)
                    v.tensor_tensor(out=wself[:], in0=wself[:],
                                    in1=nmask_sb[:, w:w + 1], op=Op.mult)
                    v.scalar_tensor_tensor(out=num[:, 0:F],
                                           in0=selfr[:, 0:F], scalar=wself[:],
                                           in1=num[:, 0:F],
                                           op0=Op.mult, op1=Op.add)
                    den = fl_pool.tile([P, 1], f32, tag="den")
                    v.tensor_tensor(out=den[:], in0=num[:, F:F + 1],
                                    in1=wself[:], op=Op.add)
                    v.tensor_scalar(out=den[:], in0=den[:], scalar1=1e-30,
                                    scalar2=None, op0=Op.max)
                    rcol = fl_pool.tile([P, 1], f32, tag="rcol")
                    v.reciprocal(out=rcol[:], in_=den[:])

                    pstr = psm_pool.tile([P, P], f32, tag="prep")
                    pe.transpose(out=pstr[:], in_=num[:, 0:F], identity=ident)
                    aggT = fl_pool.tile([P, F], f32, tag="aggT")
                    s_.activation(out=aggT[:], in_=pstr[:], func=Act.Copy)
                    psh = ps_pool.tile([P, F], f32, tag="psh")
                    pe.matmul(out=psh[:], lhsT=aggT[:], rhs=Wsb,
                              start=True, stop=True)

                    if li == 0:
                        st = stage[w % 2]
                        hv = st[:, 0:F]
                    else:
                        hs = h2st[w % 2]
                        hv = hs[:, 0:F]
                    v.scalar_tensor_tensor(out=hv, in0=psh[:], scalar=rcol[:],
                                           in1=brep, op0=Op.mult, op1=Op.add)
                    v.scalar_tensor_tensor(out=hv, in0=hv, scalar=ACT_SLOPE,
                                           in1=hv, op0=Op.mult, op1=Op.max)

                    if cfg.get("dbg") and li == 0 and w == cfg.get("dbgw", 0):
                        sy.dma_start(t["dbg_gsl"].ap(),
                                     gsl[:].rearrange("p k e -> p (k e)"))
                        sy.dma_start(t["dbg_cmpb"].ap(), cmpb[:])
                        sy.dma_start(t["dbg_dvec"].ap(), dvec[:])
                        sy.dma_start(t["dbg_logit"].ap(), logit[:])
                        sy.dma_start(t["dbg_wcol"].ap(), wcol[:])
                        sy.dma_start(t["dbg_num"].ap(), num[:])
                        sy.dma_start(t["dbg_den"].ap(), den[:])
                        sy.dma_start(t["dbg_selfr"].ap(), selfr[:])
                        sy.dma_start(t["dbg_wself"].ap(), wself[:])
                        sy.dma_start(t["dbg_hv"].ap(), hv)
                    if li == 0:
                        scol = cols_pool.tile([P, 1], f32, tag="scol")
                        v.scalar_tensor_tensor(
                            out=trash_f, in0=hv, scalar=1.0,
                            in1=a_s2, op0=Op.mult, op1=Op.mult,
                            accum_out=scol[:])
                        v.scalar_tensor_tensor(
                            out=trash_f, in0=hv, scalar=1.0,
                            in1=a_d2, op0=Op.mult, op1=Op.mult,
                            accum_out=d_next[:, w:w + 1])
                        v.tensor_copy(st[:, 130:132].bitcast(f32), scol[:])
                        shard_write(ccout, st, w)
                        for (q, a, b, o) in ag_after.get(w, ()):
                            ag_fire(ccout, T2, q, a, b, o)
                    else:
                        # fused attention pooling: scatter this window's
                        # contribution immediately instead of staging h2 in
                        # DRAM and re-reading it in a separate pass.
                        v.scalar_tensor_tensor(
                            out=trash_f, in0=hv, scalar=1.0,
                            in1=a_reps["w_attn"], op0=Op.mult, op1=Op.mult,
                            accum_out=wn_all[:, w:w + 1])
                        ewc = cols_pool.tile([P, 1], f32, tag="ewc")
                        v.tensor_tensor(out=ewc[:], in0=wn_all[:, w:w + 1],
                                        in1=battn_c, op=Op.add)
                        s_.activation(out=ewc[:], in_=ewc[:], func=Act.Exp)
                        v.tensor_tensor(out=ewc[:], in0=ewc[:],
                                        in1=nmask_sb[:, w:w + 1], op=Op.mult)
                        p0 = fl_pool.tile([P, P], f32, tag="p0")
                        v.tensor_scalar(out=p0[:], in0=iota_f,
                                        scalar1=brel_sb[:, w:w + 1],
                                        scalar2=None, op0=Op.is_equal)
                        v.tensor_scalar(out=p0[:], in0=p0[:], scalar1=ewc[:],
                                        scalar2=None, op0=Op.mult)
                        psp = ps_pool.tile([P, F + 1], f32, tag="psw")
                        pe.matmul(out=psp[:], lhsT=p0[:], rhs=hs[:, 0:F + 1],
                                  start=True, stop=True)
                        stp = scat_st[w % 2]
                        v.tensor_copy(stp[:, 0:F + 1], psp[:])
                        gp.dma_scatter_add(
                            pooled[:],
                            stp[:].rearrange("p (a e) -> p a e", a=1),
                            scat_sb[:, 8 * w:8 * w + 8], P, P, 192,
                            queue_num=w % 4)

            if "L1" in PH:
                gat_layer(0, T1, self1, cc2, W1sb, b_reps["b1"], d_all[0],
                          d_all[1], a_reps["a_src2"], a_reps["a_dst2"])
            if "L2" in PH:
                gat_layer(1, T2, cc2, None, W2sb,
                          b_reps["b2"], d_all[1], None, None, None)

            # ================= pooling =================
            if "P" not in PH:
                for tg in range(TG):
                    osb0 = tail_pool.tile([P, F], f32, tag="osb")
                    v.tensor_copy(osb0[:], zero_sb[:, 0:F])
                    sy.dma_start(out_d[tg * P:(tg + 1) * P, :], osb0[:])
                return
            if C == 1:
                gp.dma_start(pooled_r, pooled[:])
            else:
                gp.collective_compute("AllReduce", Op.add, replica_groups=rg,
                                      ins=[pooled.opt()], outs=[pooled_r.opt()])

            # ================= head =================
            pool_sum = sb("pool_sum", [P, TG * 132], f32)
            for tg in range(TG):
                pa = tail_pool.tile([P, 192], f32, tag="pa")
                pb = tail_pool.tile([P, 192], f32, tag="pb")
                sy.dma_start(pa[:], pooled_r[tg * P:(tg + 1) * P, :])
                sy.dma_start(pb[:], pooled_r[GP + tg * P:GP + (tg + 1) * P, :])
                v.tensor_tensor(out=pool_sum[:, tg * 132:tg * 132 + F + 1],
                                in0=pa[:, 0:F + 1], in1=pb[:, 0:F + 1],
                                op=Op.add)

            psz = pst_pool.tile([1, 1], f32, tag="ptiny")
            for tg in range(TG):
                pe.matmul(out=psz[:], lhsT=ones_c,
                          rhs=pool_sum[:, tg * 132 + F:tg * 132 + F + 1],
                          start=(tg == 0), stop=(tg == TG - 1))
            zsum = sb("zsum", [1, 1], f32)
            v.tensor_copy(zsum, psz[:])
            rz = sb("rz", [1, 1], f32)
            v.reciprocal(out=rz, in_=zsum)
            psrz = pst_pool.tile([P, 1], f32, tag="ptiny")
            pe.matmul(out=psrz[:], lhsT=ones_r, rhs=rz, start=True, stop=True)
            rzc = sb("rzc", [P, 1], f32)
            v.tensor_copy(rzc, psrz[:])

            for tg in range(TG):
                sc = tail_pool.tile([P, F], f32, tag="sc")
                v.tensor_scalar(out=sc[:],
                                in0=pool_sum[:, tg * 132:tg * 132 + F],
                                scalar1=rzc, scalar2=None, op0=Op.mult)
                pst = psm_pool.tile([P, P], f32, tag="prep")
                pe.transpose(out=pst[:], in_=sc[:], identity=ident)
                pT = tail_pool.tile([P, F], f32, tag="pT")
                v.tensor_copy(pT[:], pst[:])
                psz1 = ps_pool.tile([H, P], f32, tag="psh")
                pe.matmul(out=psz1[:], lhsT=fw1, rhs=pT[:], start=True,
                          stop=True)
                v.tensor_scalar(out=z_all[:, tg * P:(tg + 1) * P], in0=psz1[:],
                                scalar1=fb1, scalar2=None, op0=Op.add)

            musum_t = sb("musum_t", [H, TG], f32)
            sqsum_t = sb("sqsum_t", [H, TG], f32)
            for tg in range(TG):
                s_.activation(out=trash_z, in_=z_all[:, tg * P:(tg + 1) * P],
                              func=Act.Copy, accum_out=musum_t[:, tg:tg + 1])
                s_.activation(out=trash_z, in_=z_all[:, tg * P:(tg + 1) * P],
                              func=Act.Square, accum_out=sqsum_t[:, tg:tg + 1])
            musum = sb("musum", [H, 1], f32)
            sqsum = sb("sqsum", [H, 1], f32)
            trash_t = sb("trash_t", [H, TG], f32)
            s_.activation(out=trash_t, in_=musum_t, func=Act.Copy,
                          accum_out=musum)
            s_.activation(out=trash_t, in_=sqsum_t, func=Act.Copy,
                          accum_out=sqsum)
            mu = sb("mu", [H, 1], f32)
            v.tensor_scalar(out=mu, in0=musum, scalar1=1.0 / G, scalar2=None,
                            op0=Op.mult)
            ex2 = sb("ex2", [H, 1], f32)
            v.tensor_scalar(out=ex2, in0=sqsum, scalar1=1.0 / G, scalar2=None,
                            op0=Op.mult)
            mu2 = sb("mu2", [H, 1], f32)
            v.tensor_tensor(out=mu2, in0=mu, in1=mu, op=Op.mult)
            var = sb("var", [H, 1], f32)
            v.tensor_tensor(out=var, in0=ex2, in1=mu2, op=Op.subtract)
            v.tensor_scalar(out=var, in0=var, scalar1=BN_EPS, scalar2=None,
                            op0=Op.add)
            std = sb("std", [H, 1], f32)
            s_.activation(out=std, in_=var, func=Act.Sqrt)
            rstd = sb("rstd", [H, 1], f32)
            v.reciprocal(out=rstd, in_=std)
            gs = sb("gs", [H, 1], f32)
            v.tensor_tensor(out=gs, in0=rstd, in1=bng, op=Op.mult)

            for tg in range(TG):
                zt = tail_pool.tile([H, P], f32, tag="zt")
                v.tensor_scalar(out=zt[:], in0=z_all[:, tg * P:(tg + 1) * P],
                                scalar1=mu, scalar2=None, op0=Op.subtract)
                v.tensor_scalar(out=zt[:], in0=zt[:], scalar1=gs, scalar2=None,
                                op0=Op.mult)
                v.tensor_scalar(out=zt[:], in0=zt[:], scalar1=bnb,
                                scalar2=None, op0=Op.add)
                v.scalar_tensor_tensor(out=zt[:], in0=zt[:], scalar=ACT_SLOPE,
                                       in1=zt[:], op0=Op.mult, op1=Op.max)
                pso = ps_pool.tile([F, P], f32, tag="psh")
                pe.matmul(out=pso[:], lhsT=fw2, rhs=zt[:], start=True,
                          stop=True)
                oT = tail_pool.tile([F, P], f32, tag="oT")
                v.tensor_scalar(out=oT[:], in0=pso[:], scalar1=fb2,
                                scalar2=None, op0=Op.add)
                psf = psm_pool.tile([P, P], f32, tag="prep")
                pe.transpose(out=psf[:], in_=oT[:], identity=ident)
                osb = tail_pool.tile([P, F], f32, tag="osb")
                v.tensor_copy(osb[:], psf[:])
                sy.dma_start(out_d[tg * P:(tg + 1) * P, :], osb[:])


# --------------------------------------------------------------------------
# public entry
# --------------------------------------------------------------------------

_PROG_CACHE = {}


def _run(inputs, cfg, trace=False):
    in_maps, cfg = _host_prep(inputs, cfg)
    key = (cfg["N"], cfg["E"], cfg["G"], cfg["C"], cfg["KWmax"],
           cfg["IDXW"], cfg["nidx"])
    if key not in _PROG_CACHE:
        _PROG_CACHE[key] = _build_program(cfg)
    nc = _PROG_CACHE[key]
    res = run_bass_kernel_spmd(nc, in_maps, core_ids=list(range(cfg["C"])),
                               trace=trace)
    out = np.asarray(res.results[0]["out"], np.float32)
    return out, res


def kernel(**inputs):
    out, _ = _run(inputs, _full_cfg())
    return out



# revision 55
# speedup vs baseline: 1.1186x; 1.1186x over previous
"""Self-contained Trainium2 Bass kernel for the DrugFEM GAT model.

2-layer GATConv (heads=1, self-loops) + global-softmax attention pooling +
Linear/BatchNorm/LeakyReLU/Linear head.  Runs SPMD on 8 NeuronCores:
nodes (and their in-edges) are partitioned across cores; per-layer node
feature tables are replicated via AllGather; edge aggregation uses per
(window, quartile) indirect DMA gathers with exact valid counts (no pad
descriptors), whole-window broadcast DVE ops for the one-hot selector
build, and selector-matmul accumulation in PSUM.  Self-loop edges are
handled separately with one direct contiguous row load per window.
"""

import os
import sys

import numpy as np

for _p in ("/opt/trn_rl_repo", "/root/.axon_site/_ro/trn_rl_repo"):
    if os.path.isdir(_p) and _p not in sys.path:
        sys.path.insert(0, _p)

import ml_dtypes  # noqa: E402

from concourse import bacc, mybir, tile  # noqa: E402

# Force Exp/Ln/Copy to resolve to the one ACT func-set that contains all of
# them: emptying the narrower exp/ln sets stops bacc's table-load pass from
# thrashing tables (~1.3us per reload) inside the per-window loop.
_orig_get_act_tables = bacc.get_activation_tables


def _patched_get_act_tables(arch):
    tabs = dict(_orig_get_act_tables(arch))
    combined = None
    for name, funcs in tabs.items():
        fl = {str(f) for f in funcs}
        if any("Exp" in s for s in fl) and any("Ln" in s for s in fl):
            combined = name
            break
    if combined is not None:
        for name, funcs in list(tabs.items()):
            if name == combined:
                continue
            fl = {str(f) for f in funcs}
            if any("Exp" in s for s in fl) or any("Ln" in s for s in fl):
                tabs[name] = type(funcs)()
    return tabs


bacc.get_activation_tables = _patched_get_act_tables
from concourse.bass_utils import run_bass_kernel_spmd  # noqa: E402

BF16 = ml_dtypes.bfloat16
F = 128          # feature dim (hidden dim D == F_IN == 128)
P = 128          # partitions
ROWE = 256       # gather-table row, in bf16 elements (= 512 bytes)
Q = 4            # quartile split of the node table (int16 gather indices)
BN_EPS = 1e-5
GAT_SLOPE = 0.2
ACT_SLOPE = 0.01


def _full_cfg():
    return dict(N=100_000, E=1_600_000, G=4096, C=8)


def _r16(x):
    return (int(x) + 15) // 16 * 16


# --------------------------------------------------------------------------
# host-side sharding / metadata
# --------------------------------------------------------------------------

def _host_prep(inputs, cfg):
    N, G, C = cfg["N"], cfg["G"], cfg["C"]
    npc = N // C
    assert npc * C == N, "node count must divide core count"
    W = -(-npc // P)
    npc_pad = W * P
    assert N % Q == 0 and N // Q <= 32768 and npc % Q == 0
    npq = npc // Q

    # quartile 3's table is assembled by TWO half-AllGathers so the first
    # half can fly while the last L1 windows still compute; its row layout
    # is [concat_c shard[0:s3] ; concat_c shard[s3:npq]].
    s3 = min(npq, ((npq // 2) + 15) // 16 * 16)
    s3b = npq - s3

    ei = np.asarray(inputs["edge_index"]).astype(np.int64)
    src_a, dst_a = ei[0], ei[1]
    batch = np.asarray(inputs["batch"]).astype(np.int64)
    x = np.asarray(inputs["x"]).astype(np.float32)
    assert x.shape == (N, F)

    # host-computed layer-1 node table (saves phase A + the T1 AllGather):
    # row r of quartile q holds node (r//npq)*npc + q*npq + (r%npq), laid out
    # as [x_bf16[0:128] | col128=1.0 | cols130:131 = f32 score bytes].
    xw1 = x.astype(np.float64) @ np.asarray(inputs["W1"], np.float64)
    s1 = (xw1 @ np.asarray(inputs["a_src1"], np.float64).reshape(F)
          ).astype(np.float32)
    d1 = (xw1 @ np.asarray(inputs["a_dst1"], np.float64).reshape(F)
          ).astype(np.float32)
    Nq_ = N // Q

    def _t1_rows(nodes):
        n = len(nodes)
        rows = np.zeros((n, ROWE), BF16)
        rows[:, :F] = x[nodes].astype(BF16)
        rows[:, F] = 1.0
        bv = rows.view(np.uint8).reshape(n, 2 * ROWE)
        bv[:, 260:264] = s1[nodes].astype("<f4").view(np.uint8).reshape(-1, 4)
        return rows

    T1q_host = []
    for q in range(Q):
        r = np.arange(Nq_)
        if q == 3 and s3 < npq:
            in_h1 = r >= C * s3
            rr = np.where(in_h1, r - C * s3, r)
            sz = np.where(in_h1, s3b, s3)
            nodes = ((rr // sz) * npc + q * npq
                     + np.where(in_h1, s3 + rr % sz, rr % sz))
        else:
            nodes = (r // npq) * npc + q * npq + (r % npq)
        T1q_host.append(_t1_rows(nodes))
    assert np.bincount(batch, minlength=G).max() <= P, "graph > 128 nodes"

    order = np.argsort(dst_a, kind="stable")
    src_s, dst_s = src_a[order], dst_a[order]
    owner = dst_s // npc
    core_start = np.searchsorted(owner, np.arange(C + 1))

    # per-core edge partitions with (w, q) keys
    per_core = []
    cnt = np.zeros((C, W, Q), np.int64)
    for c in range(C):
        lo, hi = core_start[c], core_start[c + 1]
        s_c, d_c = src_s[lo:hi], dst_s[lo:hi]
        loc = d_c - c * npc
        w_c = loc // P
        q_c = (s_c % npc) // npq
        key = w_c * Q + q_c
        korder = np.argsort(key, kind="stable")
        s_c, loc, w_c, q_c = (a[korder] for a in (s_c, loc, w_c, q_c))
        key = key[korder]
        kstart = np.searchsorted(key, np.arange(W * Q + 1))
        np.add.at(cnt[c], (w_c, q_c), 1)
        per_core.append((s_c, loc, kstart))

    # shared (cross-core max) per-(w,q) sizes so the single SPMD program
    # fits every core; per-core surplus slots use idx 0 with mask 0.
    # Rounded to full 128-slot chunks so the gather writes every slot the
    # aggregation matmul reads (no per-window SBUF memsets needed).
    nidx = np.zeros((W, Q), np.int64)
    for w in range(W):
        for q in range(Q):
            c_max = int(cnt[:, w, q].max())
            nidx[w, q] = -(-c_max // P) * P if c_max else 0
    KWq = -(-nidx // P)
    qoff = np.zeros((W, Q), np.int64)
    qoff[:, 1:] = np.cumsum(KWq, axis=1)[:, :-1]
    KW_w = KWq.sum(axis=1)
    KWmax = int(KW_w.max())
    ioff = np.zeros((W, Q), np.int64)
    ioff[:, 1:] = np.cumsum(nidx // 16, axis=1)[:, :-1]
    IDXW = int((nidx // 16).sum(axis=1).max())
    rowb = 6 * KWmax

    # self-loop row segments per window: (q, row0_in_q, a, b)
    selfsegs = []
    for w in range(W):
        r0 = w * P
        rows = min(P, npc - r0)
        segs = []
        r = r0
        while r < r0 + rows:
            q = r // npq
            r_end = min((q + 1) * npq, r0 + rows)
            segs.append((int(q), int(r - q * npq), int(r - r0),
                         int(r_end - r0)))
            r = r_end
        selfsegs.append(segs)

    cfg = dict(cfg, npc=npc, W=W, npc_pad=npc_pad, npq=npq, KWmax=KWmax,
               IDXW=IDXW, rowb=rowb, s3=s3,
               nidx=tuple(map(tuple, nidx)), KWq=tuple(map(tuple, KWq)),
               qoff=tuple(map(tuple, qoff)), ioff=tuple(map(tuple, ioff)),
               KW_w=tuple(int(k) for k in KW_w),
               selfsegs=tuple(tuple(s) for s in selfsegs))

    in_maps = []
    for c in range(C):
        s_c, loc, kstart = per_core[c]
        gidx = np.zeros((W, 16, IDXW), np.int16)
        dstl = np.full((W, P, KWmax), 127.0, np.float32)
        mask = np.zeros((W, P, KWmax), np.float32)
        for w in range(W):
            for q in range(Q):
                a, b = kstart[w * Q + q], kstart[w * Q + q + 1]
                n_e = b - a
                nx = int(nidx[w, q])
                if nx == 0:
                    continue
                srcs = s_c[a:b]
                if q == 3 and s3 < npq:
                    i_in_q = (srcs % npc) % npq
                    tabrow = np.where(
                        i_in_q < s3,
                        (srcs // npc) * s3 + i_in_q,
                        C * s3 + (srcs // npc) * s3b + (i_in_q - s3))
                else:
                    tabrow = (srcs // npc) * npq + (srcs % npc) % npq
                vals = np.zeros(nx, np.int64)
                vals[:n_e] = tabrow
                wr = vals.reshape(nx // 16, 16).T.astype(np.int16)
                gidx[w, :, ioff[w, q]:ioff[w, q] + nx // 16] = wr
                i_arr = np.arange(n_e)
                cc = qoff[w, q] + i_arr // P
                pp = i_arr % P
                dstl[w, pp, cc] = (loc[a:b] - w * P).astype(np.float32)
                mask[w, pp, cc] = 1.0
        gidx = np.tile(gidx, (1, 8, 1))
        meta = np.zeros((W, P, rowb), np.uint8)
        meta[:, :, 0:2 * KWmax] = (
            dstl.astype(BF16).view(np.uint8).reshape(W, P, -1))
        meta[:, :, 2 * KWmax:6 * KWmax] = (
            mask.view(np.uint8).reshape(W, P, -1))

        # pooling metadata (unchanged from baseline)
        b_loc = batch[c * npc:(c + 1) * npc]
        brel = np.full((P, W), 500.0, np.float32)
        nmask = np.zeros((P, W), np.float32)
        scat = np.zeros((P, 8 * W), np.int16)
        GP = G + P
        for w in range(W):
            n0 = w * P
            n1 = min(n0 + P, npc)
            base = int(b_loc[n0])
            span = int(b_loc[n1 - 1]) - base + 1
            brel[: n1 - n0, w] = (b_loc[n0:n1] - base).astype(np.float32)
            nmask[: n1 - n0, w] = 1.0
            par = (w % 2) * GP
            idx = np.where(np.arange(P) < span,
                           par + base + np.arange(P),
                           par + G + np.arange(P)).astype(np.int16)
            wrapped = idx.reshape(8, 16).T
            scat[:, 8 * w:8 * w + 8] = np.tile(wrapped, (8, 1))

        d1_loc = np.zeros(npc_pad, np.float32)
        d1_loc[:npc] = d1[c * npc:(c + 1) * npc]
        m = dict(
            meta=meta,
            gidx=gidx,
            brel=brel,
            nmask=nmask,
            scat=scat,
            d1=d1_loc.reshape(W, P).T.copy(),
        )
        for q in range(Q):
            m[f"T1q{q}"] = T1q_host[q]
            m[f"self1q{q}"] = _t1_rows(
                c * npc + q * npq + np.arange(npq))
        m.update(
            W1=np.asarray(inputs["W1"], np.float32),
            W2=np.asarray(inputs["W2"], np.float32),
            a_src2=np.asarray(inputs["a_src2"], np.float32).reshape(F, 1),
            a_dst2=np.asarray(inputs["a_dst2"], np.float32).reshape(F, 1),
            b1=np.asarray(inputs["b1"], np.float32).reshape(1, F),
            b2=np.asarray(inputs["b2"], np.float32).reshape(1, F),
            w_attn=np.asarray(inputs["w_attn"], np.float32).reshape(F, 1),
            b_attn=np.asarray(inputs["b_attn"], np.float32).reshape(1, 1),
            fc_w1=np.asarray(inputs["fc_w1"], np.float32),
            fc_b1=np.asarray(inputs["fc_b1"], np.float32).reshape(-1, 1),
            bn_g=np.asarray(inputs["bn_g"], np.float32).reshape(-1, 1),
            bn_b=np.asarray(inputs["bn_b"], np.float32).reshape(-1, 1),
            fc_w2=np.asarray(inputs["fc_w2"], np.float32),
            fc_b2=np.asarray(inputs["fc_b2"], np.float32).reshape(-1, 1),
            iota_b=np.tile(np.arange(P, dtype=np.float32), (P, 1)).astype(BF16),
            iota_f=np.tile(np.arange(P, dtype=np.float32), (P, 1)),
            ident=np.eye(P, dtype=np.float32),
            ones_c=np.ones((P, 1), np.float32),
            ones_r=np.ones((1, P), np.float32),
        )
        in_maps.append(m)
    return in_maps, cfg


# --------------------------------------------------------------------------
# device program
# --------------------------------------------------------------------------

def _build_program(cfg):
    N, G, C = cfg["N"], cfg["G"], cfg["C"]
    W = cfg["W"]
    H = cfg.get("Dh", 64)          # hidden dim of the fc head (D//2)
    TG = G // P                    # graph tiles in the head
    assert G % P == 0
    GP = G + P
    rowb = cfg["rowb"]

    f32 = mybir.dt.float32
    bf16 = mybir.dt.bfloat16
    i16 = mybir.dt.int16
    u8 = mybir.dt.uint8
    Op = mybir.AluOpType
    Act = mybir.ActivationFunctionType

    nc = bacc.Bacc("TRN2", target_bir_lowering=False, debug=False,
                   enable_asserts=False, num_devices=C,
                   num_swdge_queues=4, dynamic_dma_scratch_size=65536)

    def din(name, shape, dt):
        return nc.dram_tensor(name, shape, dt, kind="ExternalInput")

    Nq = N // 4
    npq = cfg["npq"]
    t = {}
    t["meta"] = din("meta", [W, P, rowb], u8)
    t["gidx"] = din("gidx", [W, P, cfg["IDXW"]], i16)
    t["brel"] = din("brel", [P, W], f32)
    t["nmask"] = din("nmask", [P, W], f32)
    t["scat"] = din("scat", [P, 8 * W], i16)
    t["d1"] = din("d1", [P, W], f32)
    for q in range(4):
        t[f"T1q{q}"] = din(f"T1q{q}", [Nq, ROWE], bf16)
        t[f"self1q{q}"] = din(f"self1q{q}", [npq, ROWE], bf16)
    for nm, sh in [("W1", [F, F]), ("W2", [F, F]), ("fc_w1", [F, H]),
                   ("fc_w2", [H, F]),
                   ("a_src2", [F, 1]), ("a_dst2", [F, 1]), ("b1", [1, F]),
                   ("b2", [1, F]), ("w_attn", [F, 1]), ("b_attn", [1, 1]),
                   ("fc_b1", [H, 1]), ("bn_g", [H, 1]), ("bn_b", [H, 1]),
                   ("fc_b2", [F, 1]), ("iota_f", [P, P]), ("ident", [P, P]),
                   ("ones_c", [P, 1]), ("ones_r", [1, P])]:
        t[nm] = din(nm, sh, f32)
    t["iota_b"] = din("iota_b", [P, P], bf16)
    out_d = nc.dram_tensor("out", [G, F], f32, kind="ExternalOutput")
    if cfg.get("dbg"):
        KWmax = cfg["KWmax"]
        for nm, sh, dt in [("dbg_gsl", [P, KWmax * ROWE], bf16),
                           ("dbg_cmpb", [P, KWmax * P], bf16),
                           ("dbg_dvec", [P, KWmax], f32),
                           ("dbg_logit", [P, KWmax], f32),
                           ("dbg_wcol", [P, KWmax], f32),
                           ("dbg_num", [P, F + 1], f32),
                           ("dbg_den", [P, 1], f32),
                           ("dbg_selfr", [P, ROWE], bf16),
                           ("dbg_wself", [P, 1], f32),
                           ("dbg_hv", [P, F], bf16)]:
            t[nm] = nc.dram_tensor(nm, sh, dt, kind="ExternalOutput")

    with tile.TileContext(nc) as tc:
        _emit(nc, tc, t, out_d, cfg, H, TG, GP, rowb,
              f32, bf16, i16, Op, Act)
    nc.compile()
    return nc


def _emit(nc, tc, t, out_d, cfg, H, TG, GP, rowb, f32, bf16, i16, Op, Act):
    PH = cfg.get("phases", "AL1L2PT")
    N, G, C = cfg["N"], cfg["G"], cfg["C"]
    npc, W = cfg["npc"], cfg["W"]
    npq = cfg["npq"]
    KWmax, IDXW = cfg["KWmax"], cfg["IDXW"]
    nidx, KWq, qoff, ioff = cfg["nidx"], cfg["KWq"], cfg["qoff"], cfg["ioff"]
    KW_w, selfsegs = cfg["KW_w"], cfg["selfsegs"]
    Nq = N // Q
    rg = [list(range(C))]

    def sb(name, shape, dt):
        return nc.alloc_sbuf_tensor(name, list(shape), dt).ap()

    # ---- persistent sbuf ----
    iota_f = sb("iota_f_s", [P, P], f32)
    iota_b = sb("iota_b_s", [P, P], bf16)
    ident = sb("ident_s", [P, P], f32)
    ones_c = sb("ones_c_s", [P, 1], f32)
    ones_r = sb("ones_r_s", [1, P], f32)
    W1sb = sb("W1_s", [P, F], f32)
    W2sb = sb("W2_s", [P, F], f32)
    fw1 = sb("fw1_s", [F, H], f32)
    fw2 = sb("fw2_s", [H, F], f32)
    fb1 = sb("fb1_s", [H, 1], f32)
    bng = sb("bng_s", [H, 1], f32)
    bnb = sb("bnb_s", [H, 1], f32)
    fb2 = sb("fb2_s", [F, 1], f32)
    a_cols = {nm: sb(nm + "_s", [F, 1], f32)
              for nm in ("a_src2", "a_dst2", "w_attn")}
    b_rows = {nm: sb(nm + "_s", [1, F], f32) for nm in ("b1", "b2")}
    battn = sb("battn_s", [1, 1], f32)

    a_reps = {nm: sb(nm + "_rep", [P, P], f32)
              for nm in ("a_src2", "a_dst2", "w_attn")}
    b_reps = {nm: sb(nm + "_rep", [P, P], f32) for nm in ("b1", "b2")}
    battn_c = sb("battn_col", [P, 1], f32)

    d_all = [sb("d1_all", [P, W], f32), sb("d2_all", [P, W], f32)]
    wn_all = sb("wn_all", [P, W], f32)
    h2st = [sb("h2st_a", [P, 132], f32), sb("h2st_b", [P, 132], f32)]
    stage = [sb("stage_a", [P, ROWE], bf16), sb("stage_b", [P, ROWE], bf16)]
    scat_st = [sb("scat_a", [P, 192], f32), sb("scat_b", [P, 192], f32)]
    trash_f = sb("trash_f", [P, P], f32)
    n_zrow = 2 * GP * 192 // P // 192          # pooled rows per partition
    zchunk = min(n_zrow, 11)
    zero_sb = sb("zero_sb", [P, zchunk * 192], f32)
    z_all = sb("z_all", [H, G], f32)
    trash_z = sb("trash_z", [H, P], f32)

    brel_sb = sb("brel_s", [P, W], f32)
    nmask_sb = sb("nmask_s", [P, W], f32)
    scat_sb = sb("scat_s", [P, 8 * W], i16)

    v = nc.vector
    s_ = nc.scalar
    pe = nc.tensor
    gp = nc.gpsimd
    sy = nc.sync

    # ---- dram scratch ----
    # collective OUTPUTS live in Shared (pair-HBM) address space — the fast
    # path for HBM-HBM collectives; inputs must stay Local.
    T2 = [nc.dram_tensor(f"T2q{q}", [Nq, ROWE], bf16, kind="Internal",
                         addr_space="Shared").ap() for q in range(Q)]
    pooled_r = nc.dram_tensor("pooled_r", [2 * GP, 192], f32, kind="Internal",
                              addr_space="Shared").ap()
    with tc.tile_pool(name="dram", bufs=1, space="DRAM") as dpool:
        cc2 = [dpool.tile([npq, ROWE], bf16, name=f"cc2q{q}") for q in range(Q)]
        pooled = dpool.tile([2 * GP, 192], f32)

        with (
            tc.tile_pool(name="meta", bufs=3) as meta_pool,
            tc.tile_pool(name="gath", bufs=3) as gath_pool,
            tc.tile_pool(name="selfp", bufs=2) as self_pool,
            tc.tile_pool(name="cmp", bufs=2) as cmp_pool,
            tc.tile_pool(name="cols", bufs=2) as cols_pool,
            tc.tile_pool(name="fl", bufs=2) as fl_pool,
            tc.tile_pool(name="xw", bufs=2) as xw_pool,
            tc.tile_pool(name="tail", bufs=2) as tail_pool,
            tc.tile_pool(name="ps", bufs=2, space="PSUM") as ps_pool,
            tc.tile_pool(name="psm", bufs=2, space="PSUM") as psm_pool,
            tc.tile_pool(name="pst", bufs=2, space="PSUM") as pst_pool,
        ):
            # ================= prologue =================
            for nm, dest in [("iota_f", iota_f), ("iota_b", iota_b),
                             ("ident", ident), ("ones_c", ones_c),
                             ("ones_r", ones_r), ("W1", W1sb), ("W2", W2sb),
                             ("fc_w1", fw1), ("fc_w2", fw2), ("fc_b1", fb1),
                             ("bn_g", bng), ("bn_b", bnb), ("fc_b2", fb2),
                             ("brel", brel_sb), ("nmask", nmask_sb),
                             ("scat", scat_sb), ("b_attn", battn),
                             ("d1", d_all[0])]:
                sy.dma_start(dest, t[nm].ap())
            for nm in a_cols:
                sy.dma_start(a_cols[nm], t[nm].ap())
            for nm in b_rows:
                sy.dma_start(b_rows[nm], t[nm].ap())

            for hs in h2st:
                v.memset(hs[:, F:132], 1.0)
            v.memset(zero_sb, 0.0)
            for st in stage:
                v.memset(st[:, F:ROWE], 0.0)
                v.memset(st[:, F:F + 1], 1.0)
            for st in scat_st:
                v.memset(st, 0.0)
            # pre-fill the gather-pool ring once so per-window partial-chunk
            # memsets are unnecessary (stale finite data is masked out).
            for _i in range(3):
                g0 = gath_pool.tile([P, KWmax, ROWE], bf16, tag="gsl")
                v.memset(g0[:], 0.0)

            def rep_from_row(row_ap, dest):
                ps = psm_pool.tile([P, P], f32, tag="prep")
                pe.matmul(out=ps[:], lhsT=ones_r, rhs=row_ap, start=True,
                          stop=True)
                v.tensor_copy(dest, ps[:])

            def rep_from_col(col_ap, dest):
                psr = pst_pool.tile([1, P], f32, tag="ptiny")
                pe.transpose(out=psr[:], in_=col_ap, identity=ident)
                row = xw_pool.tile([1, P], f32, tag="prowsb")
                v.tensor_copy(row[:], psr[:])
                rep_from_row(row[:], dest)

            # W transposes for effective attention vectors
            WT = {}
            for nm, wsb in (("W2", W2sb),):
                pst = psm_pool.tile([P, P], f32, tag="prep")
                pe.transpose(out=pst[:], in_=wsb, identity=ident)
                wt = sb(nm + "T_s", [P, F], f32)
                v.tensor_copy(wt, pst[:])
                WT[nm] = wt
            for nm, wnm in (("a_src2", "W2"), ("a_dst2", "W2")):
                pse = pst_pool.tile([1, P], f32, tag="ptiny")
                pe.matmul(out=pse[:], lhsT=a_cols[nm], rhs=WT[wnm],
                          start=True, stop=True)
                row = xw_pool.tile([1, P], f32, tag="prowsb")
                v.tensor_copy(row[:], pse[:])
                rep_from_row(row[:], a_reps[nm])
            rep_from_col(a_cols["w_attn"], a_reps["w_attn"])
            for nm in b_rows:
                rep_from_row(b_rows[nm], b_reps[nm])
            psb = pst_pool.tile([P, 1], f32, tag="ptiny")
            pe.matmul(out=psb[:], lhsT=ones_r, rhs=battn, start=True,
                      stop=True)
            v.tensor_copy(battn_c, psb[:])

            # zero the pooled accumulators
            pv = pooled[:].rearrange("(a p) e -> p a e", p=P)
            zv = zero_sb.rearrange("p (a e) -> p a e", e=192)
            a0 = 0
            while a0 < n_zrow:
                a1 = min(a0 + zchunk, n_zrow)
                sy.dma_start(pv[:, a0:a1, :], zv[:, 0:a1 - a0, :])
                a0 = a1

            def shard_write(ccq, st, w):
                r0 = w * P
                rows = min(P, npc - r0)
                a = 0
                while a < rows:
                    q = (r0 + a) // npq
                    b = min(rows, (q + 1) * npq - r0)
                    sy.dma_start(ccq[q][r0 + a - q * npq:r0 + b - q * npq, :],
                                 st[a:b, :])
                    a = b

            # fire each quartile's AllGather as soon as its shard rows are
            # written; quartile 3 goes as two half-AllGathers so only the
            # second half's transfer is exposed after the last window.
            s3 = cfg["s3"]
            ag_after = {}
            for q in range(3):
                ag_after.setdefault(min(((q + 1) * npq - 1) // P, W - 1),
                                    []).append((q, 0, npq, 0))
            if s3 < npq:
                ag_after.setdefault(min((3 * npq + s3 - 1) // P, W - 1),
                                    []).append((3, 0, s3, 0))
                ag_after.setdefault(W - 1, []).append((3, s3, npq, C * s3))
            else:
                ag_after.setdefault(W - 1, []).append((3, 0, npq, 0))

            def ag_fire(ccq, Tq, q, a, b, o):
                if C == 1:
                    gp.dma_start(Tq[q][o:o + (b - a), :], ccq[q][a:b, :])
                else:
                    gp.collective_compute(
                        "AllGather", Op.bypass, replica_groups=rg,
                        ins=[ccq[q][a:b, :].opt()],
                        outs=[Tq[q][o:o + C * (b - a), :].opt()])

            # layer-1 table + self rows + d1 come precomputed from the host
            T1 = [t[f"T1q{q}"].ap() for q in range(Q)]
            self1 = [t[f"self1q{q}"].ap() for q in range(Q)]

            # ================= GAT layers =================
            def gat_layer(li, Tfull, ccin, ccout, Wsb, brep, d_this, d_next,
                          a_s2, a_d2):
                for w in range(W):
                    KW = KW_w[w]
                    meta = meta_pool.tile([P, rowb], mybir.dt.uint8,
                                          tag="meta")
                    sy.dma_start(meta[:], t["meta"].ap()[w])
                    dstv = meta[:, 0:2 * KWmax].bitcast(bf16)
                    maskv = meta[:, 2 * KWmax:6 * KWmax].bitcast(f32)
                    idxt = meta_pool.tile([P, IDXW], i16, tag="idxt")
                    sy.dma_start(idxt[:], t["gidx"].ap()[w])

                    gsl = gath_pool.tile([P, KWmax, ROWE], bf16, tag="gsl")
                    for q in range(Q):
                        nx = nidx[w][q]
                        if nx == 0:
                            continue
                        kq = KWq[w][q]
                        gp.dma_gather(
                            gsl[:, qoff[w][q]:qoff[w][q] + kq, :],
                            Tfull[q],
                            idxt[:, ioff[w][q]:ioff[w][q] + nx // 16],
                            nx, nx, ROWE, single_packet=False,
                            queue_num=q)

                    # self-loop rows (contiguous in the input cc tables)
                    selfr = self_pool.tile([P, ROWE], bf16, tag="selfr")
                    if selfsegs[w][-1][3] < P:
                        v.memset(selfr[:], 0.0)
                    for (q, r0q, a, b) in selfsegs[w]:
                        sy.dma_start(selfr[a:b, :], ccin[q][r0q:r0q + b - a, :])

                    # d broadcast [P, P]: row j = d_this[j, w]
                    psr = pst_pool.tile([1, P], f32, tag="ptiny")
                    pe.transpose(out=psr[:], in_=d_this[:, w:w + 1],
                                 identity=ident)
                    drow = xw_pool.tile([1, P], f32, tag="prowsb")
                    v.tensor_copy(drow[:], psr[:])
                    psd = psm_pool.tile([P, P], f32, tag="prep")
                    pe.matmul(out=psd[:], lhsT=ones_r, rhs=drow[:],
                              start=True, stop=True)
                    drep = fl_pool.tile([P, P], bf16, tag="drep")
                    s_.activation(out=drep[:], in_=psd[:], func=Act.Copy)

                    # whole-window selector build
                    cmpb = cmp_pool.tile([P, KWmax * P], bf16, tag="cmpb")
                    cmp3 = cmpb[:, 0:KW * P].rearrange("p (k j) -> p k j",
                                                       k=KW)
                    io_b = iota_b.unsqueeze(1).broadcast_to([P, KW, P])
                    dv_b = dstv[:, 0:KW].unsqueeze(2).broadcast_to([P, KW, P])
                    v.tensor_tensor(out=cmp3, in0=io_b, in1=dv_b,
                                    op=Op.is_equal)
                    tmpb = cmp_pool.tile([P, KWmax * P], bf16, tag="tmpb")
                    tmp3 = tmpb[:, 0:KW * P].rearrange("p (k j) -> p k j",
                                                       k=KW)
                    dr_b = drep[:].unsqueeze(1).broadcast_to([P, KW, P])
                    v.tensor_tensor(out=tmp3, in0=cmp3, in1=dr_b, op=Op.mult)
                    dvec = cols_pool.tile([P, KWmax], f32, tag="dvec")
                    v.tensor_reduce(out=dvec[:, 0:KW], in_=tmp3,
                                    axis=mybir.AxisListType.X, op=Op.add)

                    # per-edge logit -> weight
                    sview = gsl[:, 0:KW, 130:132].bitcast(f32).squeeze(2)
                    logit = cols_pool.tile([P, KWmax], f32, tag="logit")
                    v.tensor_tensor(out=logit[:, 0:KW], in0=dvec[:, 0:KW],
                                    in1=sview, op=Op.add)
                    v.scalar_tensor_tensor(out=logit[:, 0:KW],
                                           in0=logit[:, 0:KW],
                                           scalar=GAT_SLOPE,
                                           in1=logit[:, 0:KW],
                                           op0=Op.mult, op1=Op.max)
                    # padded slots carry real row-0 scores (bounded), so exp
                    # stays finite; the post-exp mask zeroes their weight.
                    wcol = cols_pool.tile([P, KWmax], f32, tag="wcol")
                    s_.activation(out=wcol[:, 0:KW], in_=logit[:, 0:KW],
                                  func=Act.Exp)
                    v.tensor_tensor(out=wcol[:, 0:KW], in0=wcol[:, 0:KW],
                                    in1=maskv[:, 0:KW], op=Op.mult)

                    swt = cmp_pool.tile([P, KWmax * P], bf16, tag="swt")
                    sw3 = swt[:, 0:KW * P].rearrange("p (k j) -> p k j", k=KW)
                    wc_b = wcol[:, 0:KW].unsqueeze(2).broadcast_to([P, KW, P])
                    v.tensor_tensor(out=sw3, in0=cmp3, in1=wc_b, op=Op.mult)

                    # aggregation
                    psw = ps_pool.tile([P, F + 1], f32, tag="psw")
                    for cc in range(KW):
                        pe.matmul(out=psw[:], lhsT=swt[:, cc * P:(cc + 1) * P],
                                  rhs=gsl[:, cc, 0:F + 1],
                                  start=(cc == 0), stop=(cc == KW - 1))
                    num = fl_pool.tile([P, F + 1], f32, tag="num")
                    s_.activation(out=num[:], in_=psw[:], func=Act.Copy)

                    # self-loop contribution
                    wself = cols_pool.tile([P, 1], f32, tag="wself")
                    v.tensor_tensor(out=wself[:],
                                    in0=selfr[:, 130:132].bitcast(f32),
                                    in1=d_this[:, w:w + 1], op=Op.add)
                    v.scalar_tensor_tensor(out=wself[:], in0=wself[:],
                                           scalar=GAT_SLOPE, in1=wself[:],
                                           op0=Op.mult, op1=Op.max)
                    s_.activation(out=wself[:], in_=wself[:], func=Act.Exp)
                    v.tensor_tensor(out=wself[:], in0=wself[:],
                                    in1=nmask_sb[:, w:w + 1], op=Op.mult)
                    v.scalar_tensor_tensor(out=num[:, 0:F],
                                           in0=selfr[:, 0:F], scalar=wself[:],
                                           in1=num[:, 0:F],
                                           op0=Op.mult, op1=Op.add)
                    den = fl_pool.tile([P, 1], f32, tag="den")
                    v.tensor_tensor(out=den[:], in0=num[:, F:F + 1],
                                    in1=wself[:], op=Op.add)
                    v.tensor_scalar(out=den[:], in0=den[:], scalar1=1e-30,
                                    scalar2=None, op0=Op.max)
                    rcol = fl_pool.tile([P, 1], f32, tag="rcol")
                    v.reciprocal(out=rcol[:], in_=den[:])

                    pstr = psm_pool.tile([P, P], f32, tag="prep")
                    pe.transpose(out=pstr[:], in_=num[:, 0:F], identity=ident)
                    aggT = fl_pool.tile([P, F], f32, tag="aggT")
                    s_.activation(out=aggT[:], in_=pstr[:], func=Act.Copy)
                    psh = ps_pool.tile([P, F], f32, tag="psh")
                    pe.matmul(out=psh[:], lhsT=aggT[:], rhs=Wsb,
                              start=True, stop=True)

                    if li == 0:
                        st = stage[w % 2]
                        hv = st[:, 0:F]
                    else:
                        hs = h2st[w % 2]
                        hv = hs[:, 0:F]
                    v.scalar_tensor_tensor(out=hv, in0=psh[:], scalar=rcol[:],
                                           in1=brep, op0=Op.mult, op1=Op.add)
                    v.scalar_tensor_tensor(out=hv, in0=hv, scalar=ACT_SLOPE,
                                           in1=hv, op0=Op.mult, op1=Op.max)

                    if cfg.get("dbg") and li == 0 and w == cfg.get("dbgw", 0):
                        sy.dma_start(t["dbg_gsl"].ap(),
                                     gsl[:].rearrange("p k e -> p (k e)"))
                        sy.dma_start(t["dbg_cmpb"].ap(), cmpb[:])
                        sy.dma_start(t["dbg_dvec"].ap(), dvec[:])
                        sy.dma_start(t["dbg_logit"].ap(), logit[:])
                        sy.dma_start(t["dbg_wcol"].ap(), wcol[:])
                        sy.dma_start(t["dbg_num"].ap(), num[:])
                        sy.dma_start(t["dbg_den"].ap(), den[:])
                        sy.dma_start(t["dbg_selfr"].ap(), selfr[:])
                        sy.dma_start(t["dbg_wself"].ap(), wself[:])
                        sy.dma_start(t["dbg_hv"].ap(), hv)
                    if li == 0:
                        scol = cols_pool.tile([P, 1], f32, tag="scol")
                        v.scalar_tensor_tensor(
                            out=trash_f, in0=hv, scalar=1.0,
                            in1=a_s2, op0=Op.mult, op1=Op.mult,
                            accum_out=scol[:])
                        v.scalar_tensor_tensor(
                            out=trash_f, in0=hv, scalar=1.0,
                            in1=a_d2, op0=Op.mult, op1=Op.mult,
                            accum_out=d_next[:, w:w + 1])
                        v.tensor_copy(st[:, 130:132].bitcast(f32), scol[:])
                        shard_write(ccout, st, w)
                        for (q, a, b, o) in ag_after.get(w, ()):
                            ag_fire(ccout, T2, q, a, b, o)
                    else:
                        # fused attention pooling: scatter this window's
                        # contribution immediately instead of staging h2 in
                        # DRAM and re-reading it in a separate pass.
                        v.scalar_tensor_tensor(
                            out=trash_f, in0=hv, scalar=1.0,
                            in1=a_reps["w_attn"], op0=Op.mult, op1=Op.mult,
                            accum_out=wn_all[:, w:w + 1])
                        ewc = cols_pool.tile([P, 1], f32, tag="ewc")
                        v.tensor_tensor(out=ewc[:], in0=wn_all[:, w:w + 1],
                                        in1=battn_c, op=Op.add)
                        s_.activation(out=ewc[:], in_=ewc[:], func=Act.Exp)
                        v.tensor_tensor(out=ewc[:], in0=ewc[:],
                                        in1=nmask_sb[:, w:w + 1], op=Op.mult)
                        p0 = fl_pool.tile([P, P], f32, tag="p0")
                        v.tensor_scalar(out=p0[:], in0=iota_f,
                                        scalar1=brel_sb[:, w:w + 1],
                                        scalar2=None, op0=Op.is_equal)
                        v.tensor_scalar(out=p0[:], in0=p0[:], scalar1=ewc[:],
                                        scalar2=None, op0=Op.mult)
                        psp = ps_pool.tile([P, F + 1], f32, tag="psw")
                        pe.matmul(out=psp[:], lhsT=p0[:], rhs=hs[:, 0:F + 1],
                                  start=True, stop=True)
                        stp = scat_st[w % 2]
                        v.tensor_copy(stp[:, 0:F + 1], psp[:])
                        gp.dma_scatter_add(
                            pooled[:],
                            stp[:].rearrange("p (a e) -> p a e", a=1),
                            scat_sb[:, 8 * w:8 * w + 8], P, P, 192,
                            queue_num=w % 4)

            if "L1" in PH:
                gat_layer(0, T1, self1, cc2, W1sb, b_reps["b1"], d_all[0],
                          d_all[1], a_reps["a_src2"], a_reps["a_dst2"])
            if "L2" in PH:
                gat_layer(1, T2, cc2, None, W2sb,
                          b_reps["b2"], d_all[1], None, None, None)

            # ================= pooling =================
            if "P" not in PH:
                for tg in range(TG):
                    osb0 = tail_pool.tile([P, F], f32, tag="osb")
                    v.tensor_copy(osb0[:], zero_sb[:, 0:F])
                    sy.dma_start(out_d[tg * P:(tg + 1) * P, :], osb0[:])
                return
            if C == 1:
                gp.dma_start(pooled_r, pooled[:])
            else:
                gp.collective_compute("AllReduce", Op.add, replica_groups=rg,
                                      ins=[pooled.opt()], outs=[pooled_r.opt()])

            # ================= head =================
            pool_sum = sb("pool_sum", [P, TG * 132], f32)
            for tg in range(TG):
                pa = tail_pool.tile([P, 192], f32, tag="pa")
                pb = tail_pool.tile([P, 192], f32, tag="pb")
                sy.dma_start(pa[:], pooled_r[tg * P:(tg + 1) * P, :])
                sy.dma_start(pb[:], pooled_r[GP + tg * P:GP + (tg + 1) * P, :])
                v.tensor_tensor(out=pool_sum[:, tg * 132:tg * 132 + F + 1],
                                in0=pa[:, 0:F + 1], in1=pb[:, 0:F + 1],
                                op=Op.add)

            psz = pst_pool.tile([1, 1], f32, tag="ptiny")
            for tg in range(TG):
                pe.matmul(out=psz[:], lhsT=ones_c,
                          rhs=pool_sum[:, tg * 132 + F:tg * 132 + F + 1],
                          start=(tg == 0), stop=(tg == TG - 1))
            zsum = sb("zsum", [1, 1], f32)
            v.tensor_copy(zsum, psz[:])
            rz = sb("rz", [1, 1], f32)
            v.reciprocal(out=rz, in_=zsum)
            psrz = pst_pool.tile([P, 1], f32, tag="ptiny")
            pe.matmul(out=psrz[:], lhsT=ones_r, rhs=rz, start=True, stop=True)
            rzc = sb("rzc", [P, 1], f32)
            v.tensor_copy(rzc, psrz[:])

            for tg in range(TG):
                sc = tail_pool.tile([P, F], f32, tag="sc")
                v.tensor_scalar(out=sc[:],
                                in0=pool_sum[:, tg * 132:tg * 132 + F],
                                scalar1=rzc, scalar2=None, op0=Op.mult)
                pst = psm_pool.tile([P, P], f32, tag="prep")
                pe.transpose(out=pst[:], in_=sc[:], identity=ident)
                pT = tail_pool.tile([P, F], f32, tag="pT")
                v.tensor_copy(pT[:], pst[:])
                psz1 = ps_pool.tile([H, P], f32, tag="psh")
                pe.matmul(out=psz1[:], lhsT=fw1, rhs=pT[:], start=True,
                          stop=True)
                v.tensor_scalar(out=z_all[:, tg * P:(tg + 1) * P], in0=psz1[:],
                                scalar1=fb1, scalar2=None, op0=Op.add)

            musum_t = sb("musum_t", [H, TG], f32)
            sqsum_t = sb("sqsum_t", [H, TG], f32)
            for tg in range(TG):
                s_.activation(out=trash_z, in_=z_all[:, tg * P:(tg + 1) * P],
                              func=Act.Copy, accum_out=musum_t[:, tg:tg + 1])
                s_.activation(out=trash_z, in_=z_all[:, tg * P:(tg + 1) * P],
                              func=Act.Square, accum_out=sqsum_t[:, tg:tg + 1])
            musum = sb("musum", [H, 1], f32)
            sqsum = sb("sqsum", [H, 1], f32)
            trash_t = sb("trash_t", [H, TG], f32)
            s_.activation(out=trash_t, in_=musum_t, func=Act.Copy,
                          accum_out=musum)
            s_.activation(out=trash_t, in_=sqsum_t, func=Act.Copy,
                          accum_out=sqsum)
            mu = sb("mu", [H, 1], f32)
            v.tensor_scalar(out=mu, in0=musum, scalar1=1.0 / G, scalar2=None,
                            op0=Op.mult)
            ex2 = sb("ex2", [H, 1], f32)
            v.tensor_scalar(out=ex2, in0=sqsum, scalar1=1.0 / G, scalar2=None,
                            op0=Op.mult)
            mu2 = sb("mu2", [H, 1], f32)
            v.tensor_tensor(out=mu2, in0=mu, in1=mu, op=Op.mult)
            var = sb("var", [H, 1], f32)
            v.tensor_tensor(out=var, in0=ex2, in1=mu2, op=Op.subtract)
            v.tensor_scalar(out=var, in0=var, scalar1=BN_EPS, scalar2=None,
                            op0=Op.add)
            std = sb("std", [H, 1], f32)
            s_.activation(out=std, in_=var, func=Act.Sqrt)
            rstd = sb("rstd", [H, 1], f32)
            v.reciprocal(out=rstd, in_=std)
            gs = sb("gs", [H, 1], f32)
            v.tensor_tensor(out=gs, in0=rstd, in1=bng, op=Op.mult)

            for tg in range(TG):
                zt = tail_pool.tile([H, P], f32, tag="zt")
                v.tensor_scalar(out=zt[:], in0=z_all[:, tg * P:(tg + 1) * P],
                                scalar1=mu, scalar2=None, op0=Op.subtract)
                v.tensor_scalar(out=zt[:], in0=zt[:], scalar1=gs, scalar2=None,
                                op0=Op.mult)
                v.tensor_scalar(out=zt[:], in0=zt[:], scalar1=bnb,
                                scalar2=None, op0=Op.add)
                v.scalar_tensor_tensor(out=zt[:], in0=zt[:], scalar=ACT_SLOPE,
                                       in1=zt[:], op0=Op.mult, op1=Op.max)
                pso = ps_pool.tile([F, P], f32, tag="psh")
                pe.matmul(out=pso[:], lhsT=fw2, rhs=zt[:], start=True,
                          stop=True)
                oT = tail_pool.tile([F, P], f32, tag="oT")
                v.tensor_scalar(out=oT[:], in0=pso[:], scalar1=fb2,
                                scalar2=None, op0=Op.add)
                psf = psm_pool.tile([P, P], f32, tag="prep")
                pe.transpose(out=psf[:], in_=oT[:], identity=ident)
                osb = tail_pool.tile([P, F], f32, tag="osb")
                v.tensor_copy(osb[:], psf[:])
                sy.dma_start(out_d[tg * P:(tg + 1) * P, :], osb[:])


# --------------------------------------------------------------------------
# public entry
# --------------------------------------------------------------------------

_PROG_CACHE = {}


def _run(inputs, cfg, trace=False):
    in_maps, cfg = _host_prep(inputs, cfg)
    key = (cfg["N"], cfg["E"], cfg["G"], cfg["C"], cfg["KWmax"],
           cfg["IDXW"], cfg["nidx"])
    if key not in _PROG_CACHE:
        _PROG_CACHE[key] = _build_program(cfg)
    nc = _PROG_CACHE[key]
    res = run_bass_kernel_spmd(nc, in_maps, core_ids=list(range(cfg["C"])),
                               trace=trace)
    out = np.asarray(res.results[0]["out"], np.float32)
    return out, res


def kernel(**inputs):
    out, _ = _run(inputs, _full_cfg())
    return out

